# revision 1
# baseline (speedup 1.0000x reference)
"""MoE routing kernel for Trainium2, 8 NeuronCores, expert-parallel.

V2: 3-stream error-compensated fp8 DoubleRow matmuls.

Host: gate (x @ Wg + bg), top-2 + softmax -> routing metadata only; all
expert-MLP compute, the all-to-all and the combine run on device.

Expert-parallel, bucket-padded layout: core c runs expert c. Its routed
tokens are laid out host-side in owner-bucket order with each bucket padded
to CAP slots (SR = 8*CAP columns, pad columns zero). Bucket boundaries are
then STATIC (o*CAP), so the mm2 output tile IS the all-to-all send buffer:
no permutation matmuls.

Matmuls run as fp8 DoubleRow (256-row contraction per instruction at 0.5
cyc/col -- 4x the bf16/fp32r rate) with hi/lo error compensation:
  A = Ah + Al (Ah = e4m3(A), Al = e5m2 residual -- e5m2's exponent range
  holds the small residuals exactly where e4m3 subnormals would not)
  A@B ~= Ah@Bh + Al@Bh + Ah@Bl   (the dropped Al@Bl term is ~0.1% noise)
3 DR streams per 256-rows = 1.5 cyc/col vs bf16's 2.0, with bf16-level
accuracy (verified 3.8e-3 rel err vs 4.1e-3 all-bf16).

W1/W2 are pre-scaled by 32 so their e4m3 mantissas sit in the normal range;
psum1 = 32*h, descaled inside the ACT affine args; aT is built as 32*act
(e4m3-safe: |32*act| < 240) and psum2 = 1024*y, descaled via wct.

  mm1: psum[f 128, tok] = sum_k2 {W1h (x) xh + W1h (x) xl + W1l (x) xh}
  act (Erf + Sigmoid share one ACT table set -> no table reloads; function
       choice is DATA via per-core scale/bias):
       t1 = Erf(s1*ps + bg)      even: s1=1/(32*sqrt2), bg=b1/sqrt2; odd: 0
       t2 = Sigmoid(s2*ps + bs)  even: 0 -> 0.5 (the gelu const); odd: 1/32
       v = 0.5*t1 + t2; temp = (ps + 32*b1)*v = 32*aT (bf16)
       ah = e4m3(temp) [DVE]; al = e5m2(temp - ah) [Pool]
  mm2: psum[tok 128, d 512] = sum_p2 {ah (x) W2h + al (x) W2h + ah (x) W2l}
       send tile = psum * (wcol/1024) -> bf16
  AllToAll (bf16) send -> recv, telescoped: mm2 runs in three phases over
       d_model columns (512/384/128); each phase's collective + combine
       overlap the next phase's matmuls so only the tiny last chain is
       exposed at the end
  combine: one dma_gather per phase fetches BOTH expert contributions of
       every owned token (1024 stacked int16 indices); y_shard =
       g[:TG] + g[TG:] + b2
"""

import numpy as np
import ml_dtypes

D_MODEL, D_FF, N_EXPERTS, TOP_K = 1024, 4096, 8, 2
B, S = 2, 2048
T = B * S
NCORES = 8
P = 128
SHARD = T // NCORES     # 512 tokens owned per core
FD = D_FF // P          # 32 f-tiles
FD2 = FD // 2           # 16 f-pairs
KD = D_MODEL // P       # 8 k-tiles (d_model)
KD2 = KD // 2           # 4 k-pairs
TG = SHARD // P         # 4 owned-token tiles
DH = 2                  # d_model halves (512-col matmul free dim)
MAX_CAP = 160           # SR = 8*CAP <= 1280 (SBUF residency bound)
WSCALE = 32.0           # fp8 pre-scale for W1/W2

_prog_cache = {}
_wprep_cache = {}

_bf16 = ml_dtypes.bfloat16
_e4 = ml_dtypes.float8_e4m3
_e5 = ml_dtypes.float8_e5m2


def _chunks(n, step=512):
    out = []
    o = 0
    while o < n:
        L = min(step, n - o)
        out.append((o, L))
        o += L
    return out


def _build_program(CAP):
    import concourse.tile as tile
    from concourse import bacc, mybir, library_config

    f32 = mybir.dt.float32
    bf16 = mybir.dt.bfloat16
    fp8h = mybir.dt.float8e4
    fp8l = mybir.dt.float8e5
    i16 = mybir.dt.int16
    DR = mybir.MatmulPerfMode.DoubleRow
    SR = NCORES * CAP
    G = SR // P
    CH = _chunks(SR)

    nc = bacc.Bacc("TRN2", target_bir_lowering=False, debug=False,
                   num_devices=NCORES)

    xTh = nc.dram_tensor("xTh", [P, KD2, 2, SR], fp8h, kind="ExternalInput").ap()
    xTl = nc.dram_tensor("xTl", [P, KD2, 2, SR], fp8l, kind="ExternalInput").ap()
    W1h = nc.dram_tensor("W1h", [FD // 2, P, 2, KD2 * 2 * P], fp8h,
                         kind="ExternalInput").ap()
    W1l = nc.dram_tensor("W1l", [FD // 2, P, 2, KD2 * 2 * P], fp8l,
                         kind="ExternalInput").ap()
    W2h = nc.dram_tensor("W2h", [FD2 // 2, P, 2, 2, D_MODEL], fp8h,
                         kind="ExternalInput").ap()
    W2l = nc.dram_tensor("W2l", [FD2 // 2, P, 2, 2, D_MODEL], fp8l,
                         kind="ExternalInput").ap()
    acts = nc.dram_tensor("acts", [P, 2], f32, kind="ExternalInput").ap()
    b1g = nc.dram_tensor("b1g", [P, FD], f32, kind="ExternalInput").ap()
    b1s = nc.dram_tensor("b1s", [P, FD], f32, kind="ExternalInput").ap()
    b1a = nc.dram_tensor("b1a", [P, FD], f32, kind="ExternalInput").ap()
    b2bc = nc.dram_tensor("b2bc", [P, D_MODEL], f32, kind="ExternalInput").ap()
    wct = nc.dram_tensor("wct", [P, G], f32, kind="ExternalInput").ap()
    idx12 = nc.dram_tensor("idx12", [P, 2 * SHARD // 16], i16,
                           kind="ExternalInput").ap()
    y_shard = nc.dram_tensor("y_shard", [SHARD, D_MODEL], bf16,
                             kind="ExternalOutput").ap()

    # collective payload split: a half (cols 0:512), then two quarters
    QWS = (512, 384, 128)
    QO = (0, 512, 896)
    send_q = [nc.dram_tensor(f"send_q{q}", [SR, QWS[q]], bf16).ap()
              for q in range(3)]
    recv_q = [nc.dram_tensor(f"recv_q{q}", [SR, QWS[q]], bf16).ap()
              for q in range(3)]

    with tile.TileContext(nc) as tc:
        with (
            tc.tile_pool(name="xtp", bufs=1) as xtp,
            tc.tile_pool(name="atp", bufs=1) as atp,
            tc.tile_pool(name="w1ph", bufs=3) as w1ph,
            tc.tile_pool(name="w1p", bufs=2) as w1p,
            tc.tile_pool(name="w2p", bufs=1) as w2p,
            tc.tile_pool(name="smalls", bufs=1) as smalls,
            tc.tile_pool(name="actp", bufs=3) as actp,
            tc.tile_pool(name="sndp", bufs=3) as sndp,
            tc.tile_pool(name="tmpp", bufs=2) as tmpp,
            tc.tile_pool(name="cmb", bufs=1) as cmb,
            tc.tile_pool(name="psm1", bufs=4, space="PSUM") as psm1,
            tc.tile_pool(name="psm2", bufs=4, space="PSUM") as psm2,
        ):
            nc.gpsimd.load_library(library_config.mlp)


            # x: one tile per dtype, DMAed in chunk-column slices (chunk 0
            # queued first so the PE starts early)
            xth = xtp.tile([P, KD2, 2, SR], fp8h, tag="xh", name="xh")
            xtl = xtp.tile([P, KD2, 2, SR], fp8l, tag="xl", name="xl")

            ah8, al8 = [], []
            for p2 in range(FD2):
                ah8.append(atp.tile([P, 2, SR], fp8h, tag=f"ah{p2}",
                                    name=f"ah{p2}"))
                al8.append(atp.tile([P, 2, SR], fp8l, tag=f"al{p2}",
                                    name=f"al{p2}"))

            def load_w1h(fp):
                w1fh = w1ph.tile([P, 2, KD2, 2, P], fp8h, tag="w1fh")
                nc.sync.dma_start(out=w1fh[:], in_=W1h[fp].rearrange(
                    "p c (a b q) -> p c a b q", a=KD2, b=2))
                return w1fh

            def load_w1l(fp):
                w1fl = w1p.tile([P, 2, KD2, 2, P], fp8l, tag="w1fl")
                nc.sync.dma_start(out=w1fl[:], in_=W1l[fp].rearrange(
                    "p c (a b q) -> p c a b q", a=KD2, b=2))
                return w1fl

            # FIFO prefetch: hi tiles 2 pairs ahead (pool depth 3), lo 1
            # pair ahead (depth 2) -- the hi tile is the first operand a new
            # pair's matmuls touch
            w1h_q = [load_w1h(0)]
            w1l_q = [load_w1l(0)]
            (o0, L0) = CH[0]
            nc.sync.dma_start(out=xth[:, :, :, o0:o0 + L0],
                              in_=xTh[:, :, :, o0:o0 + L0])
            nc.sync.dma_start(out=xtl[:, :, :, o0:o0 + L0],
                              in_=xTl[:, :, :, o0:o0 + L0])
            actt = smalls.tile([P, 2], f32, tag="actt")
            nc.sync.dma_start(out=actt[:], in_=acts[:, :])
            b1gt = smalls.tile([P, FD], f32, tag="b1gt")
            nc.sync.dma_start(out=b1gt[:], in_=b1g[:, :])
            b1st = smalls.tile([P, FD], f32, tag="b1st")
            nc.sync.dma_start(out=b1st[:], in_=b1s[:, :])
            b1at = smalls.tile([P, FD], f32, tag="b1at")
            nc.sync.dma_start(out=b1at[:], in_=b1a[:, :])
            for o, L in CH[1:]:
                nc.sync.dma_start(out=xth[:, :, :, o:o + L],
                                  in_=xTh[:, :, :, o:o + L])
                nc.sync.dma_start(out=xtl[:, :, :, o:o + L],
                                  in_=xTl[:, :, :, o:o + L])
            b2t = smalls.tile([P, D_MODEL], f32, tag="b2t")
            nc.sync.dma_start(out=b2t[:], in_=b2bc[:, :])
            wctt = smalls.tile([P, G], f32, tag="wctt")
            nc.sync.dma_start(out=wctt[:], in_=wct[:, :])
            ix12 = smalls.tile([P, 2 * SHARD // 16], i16, tag="ix12")
            nc.sync.dma_start(out=ix12[:], in_=idx12[:, :])

            # mm2 weights: phase A covers d_model cols 0:896 (one half + one
            # quarter collective fire at the ~75% mark; their combines
            # overlap phase B), phase B covers 768:1024. Loaded in p2-pairs
            # as background DMAs trickled through the mm1 f-loop.
            bg = []
            w2p_sets = []  # per phase: (hi list, lo list)
            PH = ((0, 512), (512, 384), (896, 128))
            for ph, (c0, W) in enumerate(PH):
                hi = [None] * FD2
                lo = [None] * FD2
                w2p_sets.append((hi, lo))
                for pp in range(FD2 // 2):
                    def _ldh(pp=pp, ph=ph, c0=c0, W=W, dt=fp8h, W2x=W2h,
                             dst=hi):
                        t = w2p.tile([P, 2, 2, W], dt, tag=f"w2{ph}h_{pp}",
                                     name=f"w2{ph}h_{pp}")
                        nc.sync.dma_start(out=t[:],
                                          in_=W2x[pp][:, :, :, c0:c0 + W])
                        dst[2 * pp] = t
                        dst[2 * pp + 1] = t
                    bg.append(_ldh)
                    def _ldl(pp=pp, ph=ph, c0=c0, W=W, dt=fp8l, W2x=W2l,
                             dst=lo):
                        t = w2p.tile([P, 2, 2, W], dt, tag=f"w2{ph}l_{pp}",
                                     name=f"w2{ph}l_{pp}")
                        nc.sync.dma_start(out=t[:],
                                          in_=W2x[pp][:, :, :, c0:c0 + W])
                        dst[2 * pp] = t
                        dst[2 * pp + 1] = t
                    bg.append(_ldl)

            def mm1_tile(f, o, L, w1ts, tmp):
                w1fh, w1fl = w1ts
                fi = f % 2
                p2, j = f // 2, f % 2
                ps = psm1.tile([P, 512], mybir.dt.float32, tag="ps1")
                n3 = 3 * KD2
                i = 0
                for (wt, xt) in ((w1fh, xth), (w1fh, xtl), (w1fl, xth)):
                    for k2 in range(KD2):
                        nc.tensor.matmul(ps[:, :L],
                                         lhsT=wt[:, fi, k2, :, :],
                                         rhs=xt[:, k2, :, o:o + L],
                                         start=(i == 0), stop=(i == n3 - 1),
                                         perf_mode=DR)
                        i += 1
                t1 = actp.tile([P, 512], bf16, tag="t1")
                t2 = actp.tile([P, 512], bf16, tag="t2")
                nc.scalar.activation(
                    t1[:, :L], ps[:, :L],
                    mybir.ActivationFunctionType.Erf,
                    bias=b1gt[:, f:f + 1], scale=actt[:, 0:1])
                nc.scalar.activation(
                    t2[:, :L], ps[:, :L],
                    mybir.ActivationFunctionType.Sigmoid,
                    bias=b1st[:, f:f + 1], scale=actt[:, 1:2])
                # v = 0.5*t1 + t2 ; temp = (ps + 32*b1) * v = 32*aT
                nc.vector.scalar_tensor_tensor(
                    out=t1[:, :L], in0=t1[:, :L], scalar=0.5,
                    in1=t2[:, :L],
                    op0=mybir.AluOpType.mult, op1=mybir.AluOpType.add)
                nc.vector.scalar_tensor_tensor(
                    out=tmp[:, j, :L], in0=ps[:, :L],
                    scalar=b1at[:, f:f + 1], in1=t1[:, :L],
                    op0=mybir.AluOpType.add, op1=mybir.AluOpType.mult)
                # hi/lo split: DVE takes a 3/8 column slice of ah, the Pool
                # takes the rest plus the al sub, BATCHED per f-pair (one
                # [P,2,*] op each) to halve the Q7 launch overhead -- both
                # engines then pace under the PE's psum production rate.
                Lh = (3 * L // 8) & ~15
                nc.vector.tensor_copy(ah8[p2][:, j, o:o + Lh],
                                      tmp[:, j, :Lh])
                if j == 1:
                    nc.gpsimd.tensor_copy(ah8[p2][:, :, o + Lh:o + L],
                                          tmp[:, :, Lh:L])
                    nc.gpsimd.tensor_sub(al8[p2][:, :, o:o + L],
                                         tmp[:, :, :L],
                                         ah8[p2][:, :, o:o + L])

            def mm2_group(g, w2ts, c0, W, q):
                w2th, w2tl = w2ts
                ps = psm2.tile([P, 512], mybir.dt.float32, tag="ps2")
                n3 = 3 * FD2
                i = 0
                for (at, wt) in ((ah8, w2th), (al8, w2th), (ah8, w2tl)):
                    for p2 in range(FD2):
                        nc.tensor.matmul(
                            ps[:, :W],
                            lhsT=at[p2][:, :, g * P:(g + 1) * P],
                            rhs=wt[p2][:, p2 % 2, :, c0:c0 + W],
                            start=(i == 0), stop=(i == n3 - 1),
                            perf_mode=DR)
                        i += 1
                snd = sndp.tile([P, 512], bf16, tag="snd")
                nc.vector.tensor_scalar_mul(snd[:, :W], ps[:, :W],
                                            wctt[:, g:g + 1])
                nc.sync.dma_start(out=send_q[q][g * P:(g + 1) * P, :],
                                  in_=snd[:, 0:W])

            # ---- mm1: f-outer (W1 loaded once per f-pair), chunk-inner
            w1h_q.append(load_w1h(1))
            for fp in range(FD // 2):
                if fp + 1 < FD // 2:
                    w1l_q.append(load_w1l(fp + 1))
                if fp + 2 < FD // 2:
                    w1h_q.append(load_w1h(fp + 2))
                w1ts = (w1h_q[0], w1l_q[0])
                for (o, L) in CH:
                    tmp = tmpp.tile([P, 2, 512], bf16, tag="tmp")
                    for j in range(2):
                        mm1_tile(2 * fp + j, o, L, w1ts, tmp)
                    if bg:
                        bg.pop(0)()
                w1h_q.pop(0)
                w1l_q.pop(0)

            while bg:
                bg.pop(0)()

            def a2a(q):
                nc.gpsimd.collective_compute(
                    "AllToAll", mybir.AluOpType.bypass,
                    replica_groups=[list(range(NCORES))],
                    ins=[send_q[q][:, :]], outs=[recv_q[q][:, :]])

            y_wrap = y_shard.rearrange("(tg p) d -> p tg d", p=P)

            def combine(q):
                # one gather fetches BOTH contributions of every owned token
                # (1024 stacked indices): halves the Pool desc-gen serial
                # cost on the critical tail
                W = QWS[q]
                g1 = cmb.tile([P, 2 * TG, W], bf16, tag=f"g1_{q}",
                              name=f"g1_{q}")
                nc.gpsimd.dma_gather(
                    out_ap=g1[:, :, :], in_ap=recv_q[q][:, :], idxs_ap=ix12[:],
                    num_idxs=2 * SHARD, num_idxs_reg=2 * SHARD, elem_size=W)
                nc.vector.tensor_add(g1[:, 0:TG, :], g1[:, 0:TG, :],
                                     g1[:, TG:2 * TG, :])
                for tg in range(TG):
                    nc.vector.tensor_add(g1[:, tg, :], g1[:, tg, :],
                                         b2t[:, QO[q]:QO[q] + W])
                nc.sync.dma_start(out=y_wrap[:, :, QO[q]:QO[q] + W],
                                  in_=g1[:, 0:TG, :])

            # ---- mm2 in three telescoping phases: each phase's collective
            # and combine overlap the next phase's matmuls; only the last
            # (128-col) chain is exposed at the end.
            for ph, (c0, W) in enumerate(PH):
                for g in range(G):
                    mm2_group(g, w2p_sets[ph], 0, W, ph)
                a2a(ph)
                combine(ph)

    nc.compile()
    return nc
    return nc


def _route(x_flat, Wg, bg):
    logits = x_flat.astype(np.float32) @ Wg.astype(np.float32) + bg
    order = np.argsort(-logits, axis=1, kind="stable")
    i1, i2 = order[:, 0], order[:, 1]
    s1 = np.take_along_axis(logits, i1[:, None], 1)[:, 0]
    s2 = np.take_along_axis(logits, i2[:, None], 1)[:, 0]
    e = np.exp((s2 - s1).astype(np.float32))
    w1 = 1.0 / (1.0 + e)
    w2 = e * w1
    return i1, i2, w1.astype(np.float32), w2.astype(np.float32)


def _hi_lo(a):
    hi = np.clip(a, -240, 240).astype(_e4)
    lo = (a - hi.astype(np.float32)).astype(_e5)
    return hi, lo


def _prep_weights(W1, W2):
    key = (id(W1), id(W2))
    hit = _wprep_cache.get(key)
    if hit is not None:
        return hit
    W1s = np.asarray(W1, np.float32) * WSCALE
    # [e, fp, p, fi, k2*2*128+...] = W1s[e, (2*k2+j)*128+p, (2*fp+fi)*128+q]
    W1r = (W1s.reshape(N_EXPERTS, KD2, 2, P, FD, P)
           .transpose(0, 4, 3, 1, 2, 5)
           .reshape(N_EXPERTS, FD // 2, 2, P, KD2 * 2 * P)
           .transpose(0, 1, 3, 2, 4))
    W1r = np.ascontiguousarray(W1r)
    W1rh, W1rl = _hi_lo(W1r)
    W2s = np.asarray(W2, np.float32) * WSCALE
    # [e, pp, p, pi, j, d] = W2s[e, (2*(2*pp+pi)+j)*128+p, d]
    W2r = (W2s.reshape(N_EXPERTS, FD2, 2, P, D_MODEL)
           .transpose(0, 1, 3, 2, 4)
           .reshape(N_EXPERTS, FD2 // 2, 2, P, 2, D_MODEL)
           .transpose(0, 1, 3, 2, 4, 5))
    W2r = np.ascontiguousarray(W2r)
    W2rh, W2rl = _hi_lo(W2r)
    _wprep_cache.clear()
    _wprep_cache[key] = (W1rh, W1rl, W2rh, W2rl)
    return _wprep_cache[key]


def _prepare(x, W1, b1, W2, b2, Wg, bg):
    x = np.asarray(x, np.float32)
    b1 = np.asarray(b1, np.float32)
    b2 = np.asarray(b2, np.float32)
    x_flat = np.ascontiguousarray(x.reshape(T, D_MODEL))
    i1, i2, w1, w2 = _route(x_flat, np.asarray(Wg, np.float32),
                            np.asarray(bg, np.float32))
    Wq = _prep_weights(W1, W2)

    jobs = {}  # expert -> (ids ascending = sorted by owner, wts)
    for e in range(N_EXPERTS):
        sel = (i1 == e) | (i2 == e)
        ids = np.nonzero(sel)[0]
        wts = np.where(i1[ids] == e, w1[ids], w2[ids]).astype(np.float32)
        jobs[e] = (ids, wts)
    return x_flat, jobs, (Wq, b1, b2)


def _wrap_idx(r):
    """[n] int -> [128, n/16] int16 (wrapped by 16, replicated 8x)."""
    n = len(r)
    w = np.zeros((16, n // 16), np.int16)
    w[np.arange(n) % 16, np.arange(n) // 16] = r
    return np.tile(w, (8, 1))


def _pass_maps(x_flat, jobs, consts, first_pass=True, strict=False):
    (W1rh, W1rl, W2rh, W2rl), b1, b2 = consts

    bucket_count = np.zeros((NCORES, NCORES), np.int64)
    for e in range(NCORES):
        ids, _ = jobs[e]
        own = ids // SHARD
        for o in range(NCORES):
            bucket_count[e, o] += (own == o).sum()
    CAP = max(16, int(-(-bucket_count.max() // 16) * 16))
    assert CAP <= MAX_CAP
    SR = NCORES * CAP
    G = SR // P

    # recv row (on the owner) of each token contribution
    src_rows = np.full((T, 2), -1, np.int64)
    slot_of = {}
    for e in range(NCORES):
        ids, _ = jobs[e]
        own = ids // SHARD
        ks = np.empty(len(ids), np.int64)
        fill = np.zeros(NCORES, np.int64)
        for o in range(NCORES):
            m = own == o
            n = int(m.sum())
            ks[m] = fill[o] + np.arange(n)
            fill[o] += n
        slot_of[e] = own * CAP + ks
        rows_recv = e * CAP + ks
        which = (src_rows[ids, 0] >= 0).astype(np.int64)
        src_rows[ids, which] = rows_recv
    if strict:
        assert (src_rows >= 0).all()

    sq2 = np.float32(1.0 / np.sqrt(2.0))
    in_maps = []
    for c in range(NCORES):
        e = c
        ids, wts = jobs[e]
        xTc = np.zeros((D_MODEL, SR), np.float32)
        wcol = np.zeros(SR, np.float32)
        if len(ids):
            slots = slot_of[e]
            xTc[:, slots] = x_flat[ids].T
            wcol[slots] = wts / np.float32(WSCALE * WSCALE)
        xh, xl = _hi_lo(xTc)
        # [p, k2, j, n] = x[(2*k2+j)*128+p, n]
        xh = np.ascontiguousarray(
            xh.reshape(KD2, 2, P, SR).transpose(2, 0, 1, 3))
        xl = np.ascontiguousarray(
            xl.reshape(KD2, 2, P, SR).transpose(2, 0, 1, 3))
        even = (e % 2 == 0)
        b1_cols = np.ascontiguousarray(b1[e].reshape(FD, P).T)  # [P, FD]
        actsel = np.zeros((P, 2), np.float32)
        actsel[:, 0] = sq2 / WSCALE if even else 0.0
        actsel[:, 1] = 0.0 if even else 1.0 / WSCALE
        b1gv = b1_cols * sq2 if even else np.zeros((P, FD), np.float32)
        b1sv = np.zeros((P, FD), np.float32) if even else b1_cols
        b2v = b2[e] if first_pass else np.zeros(D_MODEL, np.float32)
        tok0 = c * SHARD
        r1 = src_rows[tok0:tok0 + SHARD, 0]
        r2 = src_rows[tok0:tok0 + SHARD, 1]
        if not strict:
            # under npass splitting a token's two contributions may land in
            # different passes; point the missing one at a universal pad row
            # (zero on every core).
            pad_slot = _find_pad_row(bucket_count, CAP)
            r1 = np.where(r1 < 0, pad_slot, r1)
            r2 = np.where(r2 < 0, pad_slot, r2)
        in_maps.append({
            "xTh": xh, "xTl": xl,
            "W1h": W1rh[e], "W1l": W1rl[e],
            "W2h": W2rh[e], "W2l": W2rl[e],
            "acts": actsel, "b1g": np.ascontiguousarray(b1gv),
            "b1s": np.ascontiguousarray(b1sv),
            "b1a": np.ascontiguousarray(b1_cols * np.float32(WSCALE)),
            "b2bc": np.ascontiguousarray(
                np.broadcast_to(b2v, (P, D_MODEL)).astype(np.float32)),
            "wct": np.ascontiguousarray(wcol.reshape(G, P).T),
            "idx12": _wrap_idx(np.concatenate([r1, r2])),
        })
    return (CAP,), in_maps


def _find_pad_row(bucket_count, CAP):
    """Recv row index that is a zero pad slot on every core: pick (e, k)
    with k >= max_o bucket_count[e, o]."""
    per_e_max = bucket_count.max(axis=1)
    e = int(per_e_max.argmin())
    k = int(per_e_max[e])
    assert k < CAP, "no universal pad slot (all buckets full)"
    return e * CAP + k


def make_in_maps(x, W1, b1, W2, b2, Wg, bg):
    x_flat, jobs, consts = _prepare(x, W1, b1, W2, b2, Wg, bg)
    return _pass_maps(x_flat, jobs, consts, strict=True)


def get_program(key):
    if key not in _prog_cache:
        _prog_cache[key] = _build_program(*key)
    return _prog_cache[key]


def kernel(x, W1, b1, W2, b2, Wg, bg):
    from concourse.bass_utils import run_bass_kernel_spmd

    x_flat, jobs, consts = _prepare(x, W1, b1, W2, b2, Wg, bg)
    maxbucket = 0
    for e in range(N_EXPERTS):
        own = jobs[e][0] // SHARD
        if len(own):
            maxbucket = max(maxbucket, int(np.bincount(own).max()))
    npass = max(1, -(-maxbucket // MAX_CAP))
    out = None
    for p in range(npass):
        jobs_p = {e: (ids[p::npass], wts[p::npass])
                  for e, (ids, wts) in jobs.items()}
        key, in_maps = _pass_maps(x_flat, jobs_p, consts,
                                  first_pass=(p == 0), strict=(npass == 1))
        nc = get_program(key)
        res = run_bass_kernel_spmd(nc, in_maps, list(range(NCORES)))
        full = np.concatenate(
            [res.results[c]["y_shard"].astype(np.float32)
             for c in range(NCORES)], axis=0)
        out = full if out is None else out + full
    return np.ascontiguousarray(out.reshape(B, S, D_MODEL))

